# revision 9
# baseline (speedup 1.0000x reference)
"""Trainium2 Bass kernel for nn_AttnBlock_ln (dense transformer block with
self+cross attention and a channel-LayerNorm MLP).

Sharding: 8 cores = batch (2) x sequence-block (4 x 512). Each core computes
out0[b][:, blk] and out1[b][:, blk] independently; no collectives.

Per-core dataflow (bf16 matmuls with f32 PSUM accumulation; f32 residual):
  - Q/K projections in "orientation A" ([channel, n], head-major channel
    permutation applied to the weight rows host-side).
  - V projected transposed ([n, channel]) with a ones-column per head, so the
    PV matmul emits the softmax denominator as an extra output row.
  - Scores computed transposed (s^T[m, n] = k_h . q_h) so exp(scale*s) chunks
    feed PV directly as the [m-partition, n-free] operand; softmax has no
    max-subtraction (scores are tiny: |s*scale| < ~1).
  - Merge + MLP in orientation A; LayerNorm-over-channels stats via
    ones-vector matmuls (partition reduction on the PE).
"""

import os
import sys
from contextlib import ExitStack

import numpy as np
import ml_dtypes

BF16NP = ml_dtypes.bfloat16

for _p in ("/opt/trn_rl_repo",):
    if _p not in sys.path:
        sys.path.append(_p)

import concourse.bass as bass
import concourse.tile as tile
from concourse import mybir, bacc
from concourse.bass_utils import run_bass_kernel_spmd

F32 = mybir.dt.float32
F32R = mybir.dt.float32r
BF16 = mybir.dt.bfloat16
AF = mybir.ActivationFunctionType

D = 256
N = 2048
NB = 512  # per-core sequence block
H = 4
HD = 64
HDP = HD + 1  # head slot width in augmented V^T (64 v-cols + ones col)
SCALE = 1.0 / (D ** 0.5)
EPS = 1e-5
N_CORES = 8


def build_program():
    nc = bacc.Bacc()

    def din(name, shape, dt=F32R):
        return nc.dram_tensor(name, shape, dt, kind="ExternalInput")

    d0 = din("d0", [D, N], BF16)
    d1 = din("d1", [D, N], BF16)
    d0b = din("d0b", [D, NB], BF16)
    d1b = din("d1b", [D, NB], BF16)
    d0r = din("d0r", [D, NB], F32)
    d1r = din("d1r", [D, NB], F32)
    wq_t = din("wq_t", [D, D], BF16)
    wk_t = din("wk_t", [D, D], BF16)
    bqp = din("bqp", [D], F32)
    bkp = din("bkp", [D], F32)
    wv_ta = din("wv_ta", [D + 1, H * HDP], BF16)
    wm_t = din("wm_t", [D, D], BF16)
    bm = din("bm", [D], F32)
    w1_t = din("w1_t", [3 * D, 2 * D], BF16)
    b1 = din("b1", [2 * D], F32)
    g1 = din("g1", [2 * D], F32)
    be1 = din("be1", [2 * D], F32)
    w2_t = din("w2_t", [2 * D, D], BF16)
    b2 = din("b2", [D], F32)
    o0 = nc.dram_tensor("o0", [D, NB], F32, kind="ExternalOutput")
    o1 = nc.dram_tensor("o1", [D, NB], F32, kind="ExternalOutput")

    with tile.TileContext(nc) as tc, ExitStack() as ctx:
        wpool = ctx.enter_context(tc.tile_pool(name="wpool", bufs=1))
        dstream = ctx.enter_context(tc.tile_pool(name="dstream", bufs=2))
        blkpool = ctx.enter_context(tc.tile_pool(name="blkpool", bufs=1))
        qfpool = ctx.enter_context(tc.tile_pool(name="qfpool", bufs=1))
        kfpool = ctx.enter_context(tc.tile_pool(name="kfpool", bufs=1))
        vtpool = ctx.enter_context(tc.tile_pool(name="vtpool", bufs=1))
        ptpool = ctx.enter_context(tc.tile_pool(name="ptpool", bufs=2))
        xapool = ctx.enter_context(tc.tile_pool(name="xapool", bufs=2))
        xmpool = ctx.enter_context(tc.tile_pool(name="xmpool", bufs=1))
        mlppool = ctx.enter_context(tc.tile_pool(name="mlppool", bufs=2))
        scratch = ctx.enter_context(tc.tile_pool(name="scratch", bufs=4))
        strippool = ctx.enter_context(tc.tile_pool(name="strippool", bufs=2))
        outpool = ctx.enter_context(tc.tile_pool(name="outpool", bufs=1))
        ps_sc = ctx.enter_context(tc.tile_pool(name="ps_sc", bufs=2, space="PSUM"))
        ps_sm = ctx.enter_context(tc.tile_pool(name="ps_sm", bufs=4, space="PSUM"))

        # ---------------- weights / constants -> SBUF ----------------
        def ld(name, dram, shape, rearr, dt=BF16):
            t = wpool.tile(shape, dt, name=name)
            nc.sync.dma_start(t[:], dram.rearrange(rearr, p=128) if rearr else dram[:])
            return t

        wq_sb = ld("wq_sb", wq_t, [128, 2, D], "(cc p) o -> p cc o")
        wk_sb = ld("wk_sb", wk_t, [128, 2, D], "(cc p) o -> p cc o")
        wm_sb = ld("wm_sb", wm_t, [128, 2, D], "(cc p) o -> p cc o")
        w1_sb = ld("w1_sb", w1_t, [128, 6, 2 * D], "(ci p) o -> p ci o")
        w2_sb = ld("w2_sb", w2_t, [128, 4, D], "(ci p) o -> p ci o")
        wv_sb = wpool.tile([128, 2, H * HDP], BF16, name="wv_sb")
        nc.sync.dma_start(wv_sb[:], wv_ta[0:D, :].rearrange("(cc p) o -> p cc o", p=128))
        wvb_sb = wpool.tile([1, H * HDP], BF16, name="wvb_sb")
        nc.sync.dma_start(wvb_sb[:], wv_ta[D : D + 1, :])

        bq_sb = ld("bq_sb", bqp, [128, 2], "(cc p) -> p cc", F32)
        bk_sb = ld("bk_sb", bkp, [128, 2], "(cc p) -> p cc", F32)
        bm_sb = ld("bm_sb", bm, [128, 2], "(cc p) -> p cc", F32)
        b1_sb = ld("b1_sb", b1, [128, 4], "(cc p) -> p cc", F32)
        g1_sb = ld("g1_sb", g1, [128, 4], "(cc p) -> p cc", F32)
        be1_sb = ld("be1_sb", be1, [128, 4], "(cc p) -> p cc", F32)
        b2_sb = ld("b2_sb", b2, [128, 2], "(cc p) -> p cc", F32)

        ones_a = wpool.tile([128, 1], BF16, name="ones_a")
        nc.vector.memset(ones_a[:], 1.0)
        ones_bf = wpool.tile([1, 128], BF16, name="ones_bf")
        nc.vector.memset(ones_bf[:], 1.0)
        ones_b32 = wpool.tile([1, 128], F32, name="ones_b32")
        nc.vector.memset(ones_b32[:], 1.0)
        ones_b = wpool.tile([1, 128], F32R, name="ones_b")
        nc.vector.tensor_copy(ones_b[:], ones_b32[:])
        eps_sb = wpool.tile([128, 1], F32, name="eps_sb")
        nc.vector.memset(eps_sb[:], EPS)

        # block slices (per-core inputs)
        d0b_sb = blkpool.tile([128, 2, NB], BF16, name="d0b_sb")
        nc.sync.dma_start(d0b_sb[:], d0b.rearrange("(cc p) n -> p cc n", p=128))
        d1b_sb = blkpool.tile([128, 2, NB], BF16, name="d1b_sb")
        nc.sync.dma_start(d1b_sb[:], d1b.rearrange("(cc p) n -> p cc n", p=128))
        d0r_sb = blkpool.tile([128, 2, NB], F32, name="d0r_sb")
        nc.sync.dma_start(d0r_sb[:], d0r.rearrange("(cc p) n -> p cc n", p=128))
        d1r_sb = blkpool.tile([128, 2, NB], F32, name="d1r_sb")
        nc.sync.dma_start(d1r_sb[:], d1r.rearrange("(cc p) n -> p cc n", p=128))

        # ---------------- projection helpers ----------------
        def proj_a_tile(out_sb, nt, d_tile, w_sb, b_sb):
            """Orientation-A projection of one 512-col tile: out[o, nt-block]."""
            for oc in range(2):
                ps = ps_sm.tile([128, NB], F32, tag="sm")
                for cc in range(2):
                    nc.tensor.matmul(
                        ps[:],
                        w_sb[:, cc, oc * 128 : (oc + 1) * 128],
                        d_tile[:, cc, :],
                        start=(cc == 0),
                        stop=(cc == 1),
                    )
                nc.vector.tensor_scalar_add(
                    out_sb[:, oc, nt * NB : (nt + 1) * NB], ps[:], b_sb[:, oc : oc + 1]
                )

        def proj_vt_tile(vt_sb, nt, d_tile):
            """Transposed V projection for 4 n-chunks of 128 within tile nt."""
            for sub in range(4):
                n16 = nt * 4 + sub
                ps = ps_sm.tile([128, H * HDP], F32, tag="sm")
                for cc in range(2):
                    nc.tensor.matmul(
                        ps[:],
                        d_tile[:, cc, sub * 128 : (sub + 1) * 128],
                        wv_sb[:, cc, :],
                        start=(cc == 0),
                        stop=False,
                    )
                nc.tensor.matmul(
                    ps[:], ones_bf[0:1, 0:128], wvb_sb[:], start=False, stop=True
                )
                nc.vector.tensor_copy(vt_sb[:, n16, :], ps[:])

        # ---------------- desc0 projections ----------------
        q0f = qfpool.tile([128, 2, N], BF16, name="q0f")
        k0f = kfpool.tile([128, 2, N], BF16, name="k0f", tag="kf")
        v0t = vtpool.tile([128, 16, H * HDP], BF16, name="v0t")
        for nt in range(4):
            dt_ = dstream.tile([128, 2, NB], BF16, tag="dt")
            nc.sync.dma_start(
                dt_[:],
                d0.rearrange("(cc p) n -> p cc n", p=128)[:, :, nt * NB : (nt + 1) * NB],
            )
            proj_a_tile(q0f, nt, dt_, wq_sb, bq_sb)
            proj_a_tile(k0f, nt, dt_, wk_sb, bk_sb)
            proj_vt_tile(v0t, nt, dt_)

        # block projections from d0b: q0b
        q0b = blkpool.tile([128, 2, NB], BF16, name="q0b")
        proj_a_tile(q0b, 0, d0b_sb, wq_sb, bq_sb)

        # ---------------- attention template ----------------
        def attn(tag, a_full, b_blk, vt_sb, xa_sb):
            """xa_sb[:, cc, :] (f32r, [128, 2, NB]) = normalized attention out.

            a_full: [128, 2, N] f32r (lhsT side, full length: keys of softmax axis)
            b_blk:  [128, 2, NB] f32r (rhs side: this core's block)
            vt_sb:  [128, 16, H*HDP] bf16 transposed values (+ones col per head)
            """
            for hp in range(2):  # head pairs (0,1) and (2,3); pair shares cc
                cc = hp
                ptp = ptpool.tile(
                    [128, 16, 2, NB], BF16, tag="pt", name=f"pt_{tag}_{hp}"
                )
                for mc in range(16):
                    sc = ps_sc.tile([128, 2, NB], F32, tag="sc")
                    # two heads at partition offsets 0 / 64 -> concurrent
                    # row-group-tiled matmuls on the PE array
                    for i in range(2):
                        po = i * 64
                        nc.tensor.matmul(
                            sc[:, i, :],
                            a_full[po : po + 64, cc, mc * 128 : (mc + 1) * 128],
                            b_blk[po : po + 64, cc, :],
                        )
                    nc.scalar.activation(
                        ptp[:, mc, :, :], sc[:], AF.Exp, scale=SCALE
                    )
                for i in range(2):
                    h = hp * 2 + i
                    po = i * 64
                    # PV with fused denominator (ones column -> row 64)
                    pv = ps_sm.tile([128, NB], F32, tag="sm")
                    for mc in range(16):
                        nc.tensor.matmul(
                            pv[0:HDP, :],
                            vt_sb[:, mc, h * HDP : (h + 1) * HDP],
                            ptp[:, mc, i, :],
                            start=(mc == 0),
                            stop=(mc == 15),
                        )
                    strip = strippool.tile([1, NB], F32R, tag="strip")
                    nc.vector.tensor_copy(strip[:], pv[HD : HD + 1, :])
                    bc = ps_sm.tile([128, NB], F32, tag="sm")
                    nc.tensor.matmul(bc[0:64, :], ones_b[0:1, 0:64], strip[:])
                    rb = scratch.tile([64, NB], F32, tag="scr")
                    nc.vector.reciprocal(rb[:], bc[0:64, :])
                    nc.vector.tensor_mul(
                        xa_sb[po : po + 64, cc, :], pv[0:64, :], rb[:]
                    )

        def merge(xa_sb, xm_sb):
            for oc in range(2):
                ps = ps_sm.tile([128, NB], F32, tag="sm")
                for cc in range(2):
                    nc.tensor.matmul(
                        ps[:],
                        wm_sb[:, cc, oc * 128 : (oc + 1) * 128],
                        xa_sb[:, cc, :],
                        start=(cc == 0),
                        stop=(cc == 1),
                    )
                nc.vector.tensor_scalar_add(
                    xm_sb[:, oc, :], ps[:], bm_sb[:, oc : oc + 1]
                )

        # xs0 (self-attn desc0) while desc1 projections can overlap later
        xa = xapool.tile([128, 2, NB], BF16, tag="xa", name="xa_s0")
        attn("s0", k0f, q0b, v0t, xa)
        xm_s0 = xmpool.tile([128, 2, NB], BF16, name="xm_s0")
        merge(xa, xm_s0)

        # ---------------- desc1 projections (k1 reuses k0's slot) ----------------
        k1f = kfpool.tile([128, 2, N], BF16, name="k1f", tag="kf")
        v1t = vtpool.tile([128, 16, H * HDP], BF16, name="v1t")
        for nt in range(4):
            dt_ = dstream.tile([128, 2, NB], BF16, tag="dt")
            nc.sync.dma_start(
                dt_[:],
                d1.rearrange("(cc p) n -> p cc n", p=128)[:, :, nt * NB : (nt + 1) * NB],
            )
            proj_a_tile(k1f, nt, dt_, wk_sb, bk_sb)
            proj_vt_tile(v1t, nt, dt_)
        q1b = blkpool.tile([128, 2, NB], BF16, name="q1b")
        proj_a_tile(q1b, 0, d1b_sb, wq_sb, bq_sb)
        k1b = blkpool.tile([128, 2, NB], BF16, name="k1b")
        proj_a_tile(k1b, 0, d1b_sb, wk_sb, bk_sb)

        # xs1 (self-attn desc1)
        xa = xapool.tile([128, 2, NB], BF16, tag="xa", name="xa_s1")
        attn("s1", k1f, q1b, v1t, xa)
        xm_s1 = xmpool.tile([128, 2, NB], BF16, name="xm_s1")
        merge(xa, xm_s1)

        # xc0: cross attn output for desc0 block (queries q0b attend keys k1)
        xa = xapool.tile([128, 2, NB], BF16, tag="xa", name="xa_c0")
        attn("c0", k1f, q0b, v1t, xa)
        xm_c0 = xmpool.tile([128, 2, NB], BF16, name="xm_c0")
        merge(xa, xm_c0)

        # xc1: cross attn output for desc1 block (softmax over desc0 positions)
        xa = xapool.tile([128, 2, NB], BF16, tag="xa", name="xa_c1")
        attn("c1", q0f, k1b, v0t, xa)
        xm_c1 = xmpool.tile([128, 2, NB], BF16, name="xm_c1")
        merge(xa, xm_c1)

        # ---------------- MLP (per desc), split into stats / apply so the
        # sqrt and gelu ACT table sets each load only once ----------------
        def mlp_stats(dxb_sb, xm_s, xm_c, name):
            cat = [
                dxb_sb[:, 0, :], dxb_sb[:, 1, :],
                xm_s[:, 0, :], xm_s[:, 1, :],
                xm_c[:, 0, :], xm_c[:, 1, :],
            ]
            h_sb = mlppool.tile([128, 4, NB], BF16, tag="h_sb", name=f"h_{name}")
            for oc in range(4):
                ps = ps_sm.tile([128, NB], F32, tag="sm")
                for ci in range(6):
                    nc.tensor.matmul(
                        ps[:],
                        w1_sb[:, ci, oc * 128 : (oc + 1) * 128],
                        cat[ci],
                        start=(ci == 0),
                        stop=(ci == 5),
                    )
                nc.vector.tensor_scalar_add(
                    h_sb[:, oc, :], ps[:], b1_sb[:, oc : oc + 1]
                )
            # LayerNorm over the 512 channels (partition axis, via ones matmuls)
            s1p = ps_sm.tile([128, NB], F32, tag="sm")
            for oc in range(4):
                nc.tensor.matmul(
                    s1p[0:1, :], ones_a[:], h_sb[:, oc, :],
                    start=(oc == 0), stop=(oc == 3),
                )
            s2p = ps_sm.tile([128, NB], F32, tag="sm")
            for oc in range(4):
                hsq = scratch.tile([128, NB], BF16, tag="scr")
                nc.gpsimd.tensor_mul(hsq[:], h_sb[:, oc, :], h_sb[:, oc, :])
                nc.tensor.matmul(
                    s2p[0:1, :], ones_a[:], hsq[:],
                    start=(oc == 0), stop=(oc == 3),
                )
            s1 = strippool.tile([1, NB], F32R, tag="strip")
            nc.vector.tensor_scalar_mul(s1[:], s1p[0:1, :], 1.0 / (2 * D))
            s2 = strippool.tile([1, NB], F32R, tag="strip")
            nc.vector.tensor_scalar_mul(s2[:], s2p[0:1, :], 1.0 / (2 * D))
            b1p = ps_sm.tile([128, NB], F32, tag="sm")
            nc.tensor.matmul(b1p[:], ones_b[:], s1[:])
            b2p = ps_sm.tile([128, NB], F32, tag="sm")
            nc.tensor.matmul(b2p[:], ones_b[:], s2[:])
            mu = mlppool.tile([128, NB], F32, tag="mu", name=f"mu_{name}")
            nc.vector.tensor_copy(mu[:], b1p[:])
            musq = scratch.tile([128, NB], F32, tag="scr")
            nc.gpsimd.tensor_mul(musq[:], mu[:], mu[:])
            var = scratch.tile([128, NB], F32, tag="scr")
            nc.vector.tensor_sub(var[:], b2p[:], musq[:])
            std = scratch.tile([128, NB], F32, tag="scr")
            nc.scalar.activation(std[:], var[:], AF.Sqrt, bias=eps_sb[:])
            rstd = mlppool.tile([128, NB], F32, tag="rstd", name=f"rstd_{name}")
            nc.vector.reciprocal(rstd[:], std[:])
            return h_sb, mu, rstd

        def mlp_apply(h_sb, mu, rstd, dxr_sb, out_dram):
            for oc in range(4):
                xn = scratch.tile([128, NB], F32, tag="scr")
                nc.vector.tensor_sub(xn[:], h_sb[:, oc, :], mu[:])
                nc.vector.tensor_mul(xn[:], xn[:], rstd[:])
                nc.vector.tensor_scalar(
                    xn[:], xn[:],
                    g1_sb[:, oc : oc + 1], be1_sb[:, oc : oc + 1],
                    op0=mybir.AluOpType.mult, op1=mybir.AluOpType.add,
                )
                nc.scalar.activation(h_sb[:, oc, :], xn[:], AF.Gelu)
            out_sb = outpool.tile([128, 2, NB], F32, tag="out_sb")
            for oc in range(2):
                ps = ps_sm.tile([128, NB], F32, tag="sm")
                for ci in range(4):
                    nc.tensor.matmul(
                        ps[:],
                        w2_sb[:, ci, oc * 128 : (oc + 1) * 128],
                        h_sb[:, ci, :],
                        start=(ci == 0),
                        stop=(ci == 3),
                    )
                nc.vector.scalar_tensor_tensor(
                    out_sb[:, oc, :], ps[:], b2_sb[:, oc : oc + 1],
                    dxr_sb[:, oc, :],
                    op0=mybir.AluOpType.add, op1=mybir.AluOpType.add,
                )
            nc.sync.dma_start(
                out_dram.rearrange("(cc p) n -> p cc n", p=128), out_sb[:]
            )

        st0 = mlp_stats(d0b_sb, xm_s0, xm_c0, "0")
        st1 = mlp_stats(d1b_sb, xm_s1, xm_c1, "1")
        mlp_apply(*st0, d0r_sb, o0)
        mlp_apply(*st1, d1r_sb, o1)

    nc.finalize()
    return nc


def _prep_weights(Wq, bq, Wk, bk, Wv, bv, Wm, bm, W1, b1, ln_g, ln_b, W2, b2):
    f = np.float32
    perm = np.array([hd * H + h for h in range(H) for hd in range(HD)])
    wv_ta = np.zeros((D + 1, H * HDP), f)
    for h in range(H):
        rows = perm[h * HD : (h + 1) * HD]
        wv_ta[0:D, h * HDP : h * HDP + HD] = Wv[rows, :].T
        wv_ta[D, h * HDP : h * HDP + HD] = bv[rows]
        wv_ta[D, h * HDP + HD] = 1.0
    return {
        "wq_t": np.ascontiguousarray(Wq[perm, :].T).astype(BF16NP),
        "wk_t": np.ascontiguousarray(Wk[perm, :].T).astype(BF16NP),
        "bqp": np.ascontiguousarray(bq[perm], f),
        "bkp": np.ascontiguousarray(bk[perm], f),
        "wv_ta": wv_ta.astype(BF16NP),
        "wm_t": np.ascontiguousarray(Wm[:, perm].T).astype(BF16NP),
        "bm": np.ascontiguousarray(bm, f),
        "w1_t": np.ascontiguousarray(W1.T).astype(BF16NP),
        "b1": np.ascontiguousarray(b1, f),
        "g1": np.ascontiguousarray(ln_g, f),
        "be1": np.ascontiguousarray(ln_b, f),
        "w2_t": np.ascontiguousarray(W2.T).astype(BF16NP),
        "b2": np.ascontiguousarray(b2, f),
    }


def make_in_maps(desc0, desc1, weights):
    f = np.float32
    in_maps = []
    for cid in range(N_CORES):
        b, j = cid // 4, cid % 4
        s = slice(j * NB, (j + 1) * NB)
        m = dict(weights)
        m["d0"] = np.ascontiguousarray(desc0[b]).astype(BF16NP)
        m["d1"] = np.ascontiguousarray(desc1[b]).astype(BF16NP)
        m["d0b"] = np.ascontiguousarray(desc0[b][:, s]).astype(BF16NP)
        m["d1b"] = np.ascontiguousarray(desc1[b][:, s]).astype(BF16NP)
        m["d0r"] = np.ascontiguousarray(desc0[b][:, s], f)
        m["d1r"] = np.ascontiguousarray(desc1[b][:, s], f)
        in_maps.append(m)
    return in_maps


_NC_CACHE = None


def kernel(desc0, desc1, Wq, bq, Wk, bk, Wv, bv, Wm, bm, W1, b1, ln_g, ln_b, W2, b2,
           trace=False):
    global _NC_CACHE
    desc0 = np.asarray(desc0, np.float32)
    desc1 = np.asarray(desc1, np.float32)
    weights = _prep_weights(
        np.asarray(Wq, np.float32), np.asarray(bq, np.float32),
        np.asarray(Wk, np.float32), np.asarray(bk, np.float32),
        np.asarray(Wv, np.float32), np.asarray(bv, np.float32),
        np.asarray(Wm, np.float32), np.asarray(bm, np.float32),
        np.asarray(W1, np.float32), np.asarray(b1, np.float32),
        np.asarray(ln_g, np.float32), np.asarray(ln_b, np.float32),
        np.asarray(W2, np.float32), np.asarray(b2, np.float32),
    )
    if _NC_CACHE is None:
        _NC_CACHE = build_program()
    nc = _NC_CACHE
    in_maps = make_in_maps(desc0, desc1, weights)
    res = run_bass_kernel_spmd(nc, in_maps, core_ids=list(range(N_CORES)), trace=trace)
    B = desc0.shape[0]
    out0 = np.empty((B, D, N), np.float32)
    out1 = np.empty((B, D, N), np.float32)
    for cid in range(N_CORES):
        b, j = cid // 4, cid % 4
        s = slice(j * NB, (j + 1) * NB)
        out0[b][:, s] = res.results[cid]["o0"]
        out1[b][:, s] = res.results[cid]["o1"]
    if trace:
        kernel.last_exec_time_ns = res.exec_time_ns
    return out0, out1


# revision 12
# speedup vs baseline: 1.0679x; 1.0679x over previous
"""Trainium2 Bass kernel for nn_AttnBlock_ln (dense transformer block with
self+cross attention and a channel-LayerNorm MLP).

Sharding: 8 cores = batch (2) x sequence-block (4 x 512). Each core computes
out0[b][:, blk] and out1[b][:, blk] independently; no collectives.

Per-core dataflow (bf16 matmuls with f32 PSUM accumulation; f32 residual):
  - Q/K projections in "orientation A" ([channel, n], head-major channel
    permutation applied to the weight rows host-side).
  - V projected transposed ([n, channel]) with a ones-column per head, so the
    PV matmul emits the softmax denominator as an extra output row.
  - Scores computed transposed (s^T[m, n] = k_h . q_h) so exp(scale*s) chunks
    feed PV directly as the [m-partition, n-free] operand; softmax has no
    max-subtraction (scores are tiny: |s*scale| < ~1).
  - Merge + MLP in orientation A; LayerNorm-over-channels stats via
    ones-vector matmuls (partition reduction on the PE).
"""

import os
import sys
from contextlib import ExitStack

import numpy as np
import ml_dtypes

BF16NP = ml_dtypes.bfloat16

for _p in ("/opt/trn_rl_repo",):
    if _p not in sys.path:
        sys.path.append(_p)

import concourse.bass as bass
import concourse.tile as tile
from concourse import mybir, bacc
from concourse.bass_utils import run_bass_kernel_spmd

F32 = mybir.dt.float32
F32R = mybir.dt.float32r
BF16 = mybir.dt.bfloat16
AF = mybir.ActivationFunctionType

D = 256
N = 2048
NB = 512  # per-core sequence block
H = 4
HD = 64
HDP = HD + 1  # head slot width in augmented V^T (64 v-cols + ones col)
SCALE = 1.0 / (D ** 0.5)
EPS = 1e-5
N_CORES = 8


def build_program():
    nc = bacc.Bacc()

    def din(name, shape, dt=F32R):
        return nc.dram_tensor(name, shape, dt, kind="ExternalInput")

    d0 = din("d0", [D, N], BF16)
    d1 = din("d1", [D, N], BF16)
    d0b = din("d0b", [D, NB], BF16)
    d1b = din("d1b", [D, NB], BF16)
    d0r = din("d0r", [D, NB], F32)
    d1r = din("d1r", [D, NB], F32)
    wq_t = din("wq_t", [D, D], BF16)
    wk_t = din("wk_t", [D, D], BF16)
    bqp = din("bqp", [D], F32)
    bkp = din("bkp", [D], F32)
    wv_ta = din("wv_ta", [D + 1, H * HDP], BF16)
    wm_t = din("wm_t", [D, D], BF16)
    bm = din("bm", [D], F32)
    w1_t = din("w1_t", [3 * D, 2 * D], BF16)
    b1 = din("b1", [2 * D], F32)
    g1 = din("g1", [2 * D], F32)
    be1 = din("be1", [2 * D], F32)
    w2_t = din("w2_t", [2 * D, D], BF16)
    b2 = din("b2", [D], F32)
    o0 = nc.dram_tensor("o0", [D, NB], F32, kind="ExternalOutput")
    o1 = nc.dram_tensor("o1", [D, NB], F32, kind="ExternalOutput")

    with tile.TileContext(nc) as tc, ExitStack() as ctx:
        wpool = ctx.enter_context(tc.tile_pool(name="wpool", bufs=1))
        dstream = ctx.enter_context(tc.tile_pool(name="dstream", bufs=8))
        blkpool = ctx.enter_context(tc.tile_pool(name="blkpool", bufs=1))
        qfpool = ctx.enter_context(tc.tile_pool(name="qfpool", bufs=1))
        kfpool = ctx.enter_context(tc.tile_pool(name="kfpool", bufs=1))
        vtpool = ctx.enter_context(tc.tile_pool(name="vtpool", bufs=1))
        ptpool = ctx.enter_context(tc.tile_pool(name="ptpool", bufs=2))
        xapool = ctx.enter_context(tc.tile_pool(name="xapool", bufs=2))
        xmpool = ctx.enter_context(tc.tile_pool(name="xmpool", bufs=1))
        mlppool = ctx.enter_context(tc.tile_pool(name="mlppool", bufs=2))
        scratch = ctx.enter_context(tc.tile_pool(name="scratch", bufs=4))
        strippool = ctx.enter_context(tc.tile_pool(name="strippool", bufs=2))
        outpool = ctx.enter_context(tc.tile_pool(name="outpool", bufs=1))
        ps_sc = ctx.enter_context(tc.tile_pool(name="ps_sc", bufs=2, space="PSUM"))
        ps_sm = ctx.enter_context(tc.tile_pool(name="ps_sm", bufs=4, space="PSUM"))

        # ---------------- weights / constants -> SBUF ----------------
        def ld(name, dram, shape, rearr, dt=BF16):
            t = wpool.tile(shape, dt, name=name)
            nc.sync.dma_start(t[:], dram.rearrange(rearr, p=128) if rearr else dram[:])
            return t

        wq_sb = ld("wq_sb", wq_t, [128, 2, D], "(cc p) o -> p cc o")
        wk_sb = ld("wk_sb", wk_t, [128, 2, D], "(cc p) o -> p cc o")
        wm_sb = ld("wm_sb", wm_t, [128, 2, D], "(cc p) o -> p cc o")
        w1_sb = ld("w1_sb", w1_t, [128, 6, 2 * D], "(ci p) o -> p ci o")
        w2_sb = ld("w2_sb", w2_t, [128, 4, D], "(ci p) o -> p ci o")
        wv_sb = wpool.tile([128, 2, H * HDP], BF16, name="wv_sb")
        nc.sync.dma_start(wv_sb[:], wv_ta[0:D, :].rearrange("(cc p) o -> p cc o", p=128))
        wvb_sb = wpool.tile([1, H * HDP], BF16, name="wvb_sb")
        nc.sync.dma_start(wvb_sb[:], wv_ta[D : D + 1, :])

        bq_sb = ld("bq_sb", bqp, [128, 2], "(cc p) -> p cc", F32)
        bk_sb = ld("bk_sb", bkp, [128, 2], "(cc p) -> p cc", F32)
        bm_sb = ld("bm_sb", bm, [128, 2], "(cc p) -> p cc", F32)
        b1_sb = ld("b1_sb", b1, [128, 4], "(cc p) -> p cc", F32)
        g1_sb = ld("g1_sb", g1, [128, 4], "(cc p) -> p cc", F32)
        be1_sb = ld("be1_sb", be1, [128, 4], "(cc p) -> p cc", F32)
        b2_sb = ld("b2_sb", b2, [128, 2], "(cc p) -> p cc", F32)

        ones_a = wpool.tile([128, 1], BF16, name="ones_a")
        nc.vector.memset(ones_a[:], 1.0)
        ones_bf = wpool.tile([1, 128], BF16, name="ones_bf")
        nc.vector.memset(ones_bf[:], 1.0)
        ones_b32 = wpool.tile([1, 128], F32, name="ones_b32")
        nc.vector.memset(ones_b32[:], 1.0)
        ones_b = wpool.tile([1, 128], F32R, name="ones_b")
        nc.vector.tensor_copy(ones_b[:], ones_b32[:])
        eps_sb = wpool.tile([128, 1], F32, name="eps_sb")
        nc.vector.memset(eps_sb[:], EPS)


        # block slices (per-core inputs); residual copies are DMA'd late
        d0b_sb = blkpool.tile([128, 2, NB], BF16, name="d0b_sb")
        nc.sync.dma_start(d0b_sb[:], d0b.rearrange("(cc p) n -> p cc n", p=128))

        # ---------------- projection helpers ----------------
        def proj_a_tile(out_sb, nt, d_tile, w_sb, b_sb):
            """Orientation-A projection of one 512-col tile: out[o, nt-block]."""
            for oc in range(2):
                ps = ps_sm.tile([128, NB], F32, tag="sm")
                for cc in range(2):
                    nc.tensor.matmul(
                        ps[:],
                        w_sb[:, cc, oc * 128 : (oc + 1) * 128],
                        d_tile[:, cc, :],
                        start=(cc == 0),
                        stop=(cc == 1),
                    )
                nc.vector.tensor_scalar_add(
                    out_sb[:, oc, nt * NB : (nt + 1) * NB], ps[:], b_sb[:, oc : oc + 1]
                )

        def proj_vt_tile(vt_sb, nt, d_tile):
            """Transposed V projection for 4 n-chunks of 128 within tile nt."""
            for sub in range(4):
                n16 = nt * 4 + sub
                ps = ps_sm.tile([128, H * HDP], F32, tag="sm")
                for cc in range(2):
                    nc.tensor.matmul(
                        ps[:],
                        d_tile[:, cc, sub * 128 : (sub + 1) * 128],
                        wv_sb[:, cc, :],
                        start=(cc == 0),
                        stop=False,
                    )
                nc.tensor.matmul(
                    ps[:], ones_bf[0:1, 0:128], wvb_sb[:], start=False, stop=True
                )
                nc.vector.tensor_copy(vt_sb[:, n16, :], ps[:])

        # ---------------- attention template (split: scores+exp / pv) -------
        def attn_scores(tag, a_full, b_blk):
            ptps = []
            for hp in range(2):  # head pairs (0,1), (2,3); pair shares cc
                cc = hp
                ptp = ptpool.tile(
                    [128, 16, 2, NB], BF16, tag="pt", name=f"pt_{tag}_{hp}"
                )
                for mc in range(16):
                    sc = ps_sc.tile([128, 2, NB], F32, tag="sc")
                    # two heads at partition offsets 0 / 64 -> concurrent
                    # row-group-tiled matmuls on the PE array
                    for i in range(2):
                        po = i * 64
                        nc.tensor.matmul(
                            sc[:, i, :],
                            a_full[po : po + 64, cc, mc * 128 : (mc + 1) * 128],
                            b_blk[po : po + 64, cc, :],
                        )
                    nc.scalar.activation(
                        ptp[:, mc, :, :], sc[:], AF.Exp, scale=SCALE
                    )
                ptps.append(ptp)
            return ptps

        def attn_pv(ptps, vt_sb, xa_sb):
            for hp in range(2):
                cc = hp
                ptp = ptps[hp]
                for i in range(2):
                    h = hp * 2 + i
                    po = i * 64
                    # PV with fused denominator (ones column -> row 64)
                    pv = ps_sm.tile([128, NB], F32, tag="sm")
                    for mc in range(16):
                        nc.tensor.matmul(
                            pv[0:HDP, :],
                            vt_sb[:, mc, h * HDP : (h + 1) * HDP],
                            ptp[:, mc, i, :],
                            start=(mc == 0),
                            stop=(mc == 15),
                        )
                    strip = strippool.tile([1, NB], F32R, tag="strip")
                    nc.vector.tensor_copy(strip[:], pv[HD : HD + 1, :])
                    bc = ps_sm.tile([128, NB], F32, tag="sm")
                    nc.tensor.matmul(bc[0:64, :], ones_b[0:1, 0:64], strip[:])
                    rb = scratch.tile([64, NB], F32, tag="scr")
                    nc.vector.reciprocal(rb[:], bc[0:64, :])
                    nc.vector.tensor_mul(
                        xa_sb[po : po + 64, cc, :], pv[0:64, :], rb[:]
                    )

        def merge(xa_sb, xm_sb):
            for oc in range(2):
                ps = ps_sm.tile([128, NB], F32, tag="sm")
                for cc in range(2):
                    nc.tensor.matmul(
                        ps[:],
                        wm_sb[:, cc, oc * 128 : (oc + 1) * 128],
                        xa_sb[:, cc, :],
                        start=(cc == 0),
                        stop=(cc == 1),
                    )
                nc.vector.tensor_scalar_add(
                    xm_sb[:, oc, :], ps[:], bm_sb[:, oc : oc + 1]
                )

        # ================= schedule =================
        # 1) k0f + q0b first so the softmax pipeline (ACT) starts ASAP;
        #    d0 tiles are held for the later q0f / v0t projections.
        k0f = kfpool.tile([128, 2, N], BF16, name="k0f", tag="kf")
        d0_tiles = []
        for nt in range(4):
            dt_ = dstream.tile([128, 2, NB], BF16, tag="dt", name=f"d0t{nt}")
            nc.sync.dma_start(
                dt_[:],
                d0.rearrange("(cc p) n -> p cc n", p=128)[:, :, nt * NB : (nt + 1) * NB],
            )
            d0_tiles.append(dt_)
            proj_a_tile(k0f, nt, dt_, wk_sb, bk_sb)
        q0b = blkpool.tile([128, 2, NB], BF16, name="q0b")
        proj_a_tile(q0b, 0, d0b_sb, wq_sb, bq_sb)

        # 2) self-attn desc0 scores; v0t projections fill the PE meanwhile
        pt_s0 = attn_scores("s0", k0f, q0b)
        v0t = vtpool.tile([128, 16, H * HDP], BF16, name="v0t")
        for nt in range(4):
            proj_vt_tile(v0t, nt, d0_tiles[nt])
        xa_s0 = xapool.tile([128, 2, NB], BF16, tag="xa", name="xa_s0")
        attn_pv(pt_s0, v0t, xa_s0)
        xm_s0 = xmpool.tile([128, 2, NB], BF16, name="xm_s0")
        merge(xa_s0, xm_s0)

        # 3) desc1 projections (k1 reuses k0's slot once s0 scores are done)
        d1b_sb = blkpool.tile([128, 2, NB], BF16, name="d1b_sb")
        nc.sync.dma_start(d1b_sb[:], d1b.rearrange("(cc p) n -> p cc n", p=128))
        k1f = kfpool.tile([128, 2, N], BF16, name="k1f", tag="kf")
        d1_tiles = []
        for nt in range(4):
            dt_ = dstream.tile([128, 2, NB], BF16, tag="dt", name=f"d1t{nt}")
            nc.sync.dma_start(
                dt_[:],
                d1.rearrange("(cc p) n -> p cc n", p=128)[:, :, nt * NB : (nt + 1) * NB],
            )
            d1_tiles.append(dt_)
            proj_a_tile(k1f, nt, dt_, wk_sb, bk_sb)
        q1b = blkpool.tile([128, 2, NB], BF16, name="q1b")
        proj_a_tile(q1b, 0, d1b_sb, wq_sb, bq_sb)
        k1b = blkpool.tile([128, 2, NB], BF16, name="k1b")
        proj_a_tile(k1b, 0, d1b_sb, wk_sb, bk_sb)

        # 4) cross-attn for desc0 block (c0) first so mlp0 can start early
        pt_c0 = attn_scores("c0", k1f, q0b)
        v1t = vtpool.tile([128, 16, H * HDP], BF16, name="v1t")
        for nt in range(4):
            proj_vt_tile(v1t, nt, d1_tiles[nt])
        xa_c0 = xapool.tile([128, 2, NB], BF16, tag="xa", name="xa_c0")
        attn_pv(pt_c0, v1t, xa_c0)
        xm_c0 = xmpool.tile([128, 2, NB], BF16, name="xm_c0")
        merge(xa_c0, xm_c0)
        # ---------------- MLP (per desc), split into stats / apply so the
        # sqrt and gelu ACT table sets each load only once ----------------
        def mlp_stats(dxb_sb, xm_s, xm_c, name):
            cat = [
                dxb_sb[:, 0, :], dxb_sb[:, 1, :],
                xm_s[:, 0, :], xm_s[:, 1, :],
                xm_c[:, 0, :], xm_c[:, 1, :],
            ]
            h_sb = mlppool.tile([128, 4, NB], BF16, tag="h_sb", name=f"h_{name}")
            for oc in range(4):
                ps = ps_sm.tile([128, NB], F32, tag="sm")
                for ci in range(6):
                    nc.tensor.matmul(
                        ps[:],
                        w1_sb[:, ci, oc * 128 : (oc + 1) * 128],
                        cat[ci],
                        start=(ci == 0),
                        stop=(ci == 5),
                    )
                nc.vector.tensor_scalar_add(
                    h_sb[:, oc, :], ps[:], b1_sb[:, oc : oc + 1]
                )
            # LayerNorm over the 512 channels (partition axis, via ones matmuls)
            s1p = ps_sm.tile([128, NB], F32, tag="sm")
            for oc in range(4):
                nc.tensor.matmul(
                    s1p[0:1, :], ones_a[:], h_sb[:, oc, :],
                    start=(oc == 0), stop=(oc == 3),
                )
            s2p = ps_sm.tile([128, NB], F32, tag="sm")
            for oc in range(4):
                hsq = scratch.tile([128, NB], BF16, tag="scr")
                nc.gpsimd.tensor_mul(hsq[:], h_sb[:, oc, :], h_sb[:, oc, :])
                nc.tensor.matmul(
                    s2p[0:1, :], ones_a[:], hsq[:],
                    start=(oc == 0), stop=(oc == 3),
                )
            s1 = strippool.tile([1, NB], F32R, tag="strip")
            nc.vector.tensor_scalar_mul(s1[:], s1p[0:1, :], 1.0 / (2 * D))
            s2 = strippool.tile([1, NB], F32R, tag="strip")
            nc.vector.tensor_scalar_mul(s2[:], s2p[0:1, :], 1.0 / (2 * D))
            b1p = ps_sm.tile([128, NB], F32, tag="sm")
            nc.tensor.matmul(b1p[:], ones_b[:], s1[:])
            b2p = ps_sm.tile([128, NB], F32, tag="sm")
            nc.tensor.matmul(b2p[:], ones_b[:], s2[:])
            mu = mlppool.tile([128, NB], F32, tag="mu", name=f"mu_{name}")
            nc.vector.tensor_copy(mu[:], b1p[:])
            musq = scratch.tile([128, NB], F32, tag="scr")
            nc.gpsimd.tensor_mul(musq[:], mu[:], mu[:])
            var = scratch.tile([128, NB], F32, tag="scr")
            nc.vector.tensor_sub(var[:], b2p[:], musq[:])
            lnv = scratch.tile([128, NB], F32, tag="scr")
            nc.scalar.activation(lnv[:], var[:], AF.Ln, bias=eps_sb[:])
            rstd = mlppool.tile([128, NB], F32, tag="rstd", name=f"rstd_{name}")
            nc.scalar.activation(rstd[:], lnv[:], AF.Exp, scale=-0.5)
            return h_sb, mu, rstd

        def mlp_apply(h_sb, mu, rstd, dxr_sb, out_dram):
            for oc in range(4):
                xn = scratch.tile([128, NB], F32, tag="scr")
                nc.vector.tensor_sub(xn[:], h_sb[:, oc, :], mu[:])
                nc.vector.tensor_mul(xn[:], xn[:], rstd[:])
                nc.vector.tensor_scalar(
                    xn[:], xn[:],
                    g1_sb[:, oc : oc + 1], be1_sb[:, oc : oc + 1],
                    op0=mybir.AluOpType.mult, op1=mybir.AluOpType.add,
                )
                nc.scalar.activation(h_sb[:, oc, :], xn[:], AF.Gelu)
            out_sb = outpool.tile([128, 2, NB], F32, tag="out_sb")
            for oc in range(2):
                ps = ps_sm.tile([128, NB], F32, tag="sm")
                for ci in range(4):
                    nc.tensor.matmul(
                        ps[:],
                        w2_sb[:, ci, oc * 128 : (oc + 1) * 128],
                        h_sb[:, ci, :],
                        start=(ci == 0),
                        stop=(ci == 3),
                    )
                nc.vector.scalar_tensor_tensor(
                    out_sb[:, oc, :], ps[:], b2_sb[:, oc : oc + 1],
                    dxr_sb[:, oc, :],
                    op0=mybir.AluOpType.add, op1=mybir.AluOpType.add,
                )
            nc.sync.dma_start(
                out_dram.rearrange("(cc p) n -> p cc n", p=128), out_sb[:]
            )

        # 5) mlp0 stats (conv1 + LN stats) — overlaps following attention
        st0 = mlp_stats(d0b_sb, xm_s0, xm_c0, "0")

        # 6) self-attn desc1
        xa_s1 = xapool.tile([128, 2, NB], BF16, tag="xa", name="xa_s1")
        attn_pv(attn_scores("s1", k1f, q1b), v1t, xa_s1)
        xm_s1 = xmpool.tile([128, 2, NB], BF16, name="xm_s1")
        merge(xa_s1, xm_s1)

        # 7) q0f projection (held d0 tiles) for the c1 scores
        q0f = qfpool.tile([128, 2, N], BF16, name="q0f")
        for nt in range(4):
            proj_a_tile(q0f, nt, d0_tiles[nt], wq_sb, bq_sb)

        # 8) cross-attn for desc1 block (softmax over desc0 positions)
        xa_c1 = xapool.tile([128, 2, NB], BF16, tag="xa", name="xa_c1")
        attn_pv(attn_scores("c1", q0f, k1b), v0t, xa_c1)
        xm_c1 = xmpool.tile([128, 2, NB], BF16, name="xm_c1")
        merge(xa_c1, xm_c1)

        # 9) mlp1 stats, then both applies (gelu table loads once)
        st1 = mlp_stats(d1b_sb, xm_s1, xm_c1, "1")
        d0r_sb = blkpool.tile([128, 2, NB], F32, name="d0r_sb")
        nc.sync.dma_start(d0r_sb[:], d0r.rearrange("(cc p) n -> p cc n", p=128))
        d1r_sb = blkpool.tile([128, 2, NB], F32, name="d1r_sb")
        nc.sync.dma_start(d1r_sb[:], d1r.rearrange("(cc p) n -> p cc n", p=128))
        mlp_apply(*st0, d0r_sb, o0)
        mlp_apply(*st1, d1r_sb, o1)

    nc.finalize()
    return nc


def _prep_weights(Wq, bq, Wk, bk, Wv, bv, Wm, bm, W1, b1, ln_g, ln_b, W2, b2):
    f = np.float32
    perm = np.array([hd * H + h for h in range(H) for hd in range(HD)])
    wv_ta = np.zeros((D + 1, H * HDP), f)
    for h in range(H):
        rows = perm[h * HD : (h + 1) * HD]
        wv_ta[0:D, h * HDP : h * HDP + HD] = Wv[rows, :].T
        wv_ta[D, h * HDP : h * HDP + HD] = bv[rows]
        wv_ta[D, h * HDP + HD] = 1.0
    return {
        "wq_t": np.ascontiguousarray(Wq[perm, :].T).astype(BF16NP),
        "wk_t": np.ascontiguousarray(Wk[perm, :].T).astype(BF16NP),
        "bqp": np.ascontiguousarray(bq[perm], f),
        "bkp": np.ascontiguousarray(bk[perm], f),
        "wv_ta": wv_ta.astype(BF16NP),
        "wm_t": np.ascontiguousarray(Wm[:, perm].T).astype(BF16NP),
        "bm": np.ascontiguousarray(bm, f),
        "w1_t": np.ascontiguousarray(W1.T).astype(BF16NP),
        "b1": np.ascontiguousarray(b1, f),
        "g1": np.ascontiguousarray(ln_g, f),
        "be1": np.ascontiguousarray(ln_b, f),
        "w2_t": np.ascontiguousarray(W2.T).astype(BF16NP),
        "b2": np.ascontiguousarray(b2, f),
    }


def make_in_maps(desc0, desc1, weights):
    f = np.float32
    in_maps = []
    for cid in range(N_CORES):
        b, j = cid // 4, cid % 4
        s = slice(j * NB, (j + 1) * NB)
        m = dict(weights)
        m["d0"] = np.ascontiguousarray(desc0[b]).astype(BF16NP)
        m["d1"] = np.ascontiguousarray(desc1[b]).astype(BF16NP)
        m["d0b"] = np.ascontiguousarray(desc0[b][:, s]).astype(BF16NP)
        m["d1b"] = np.ascontiguousarray(desc1[b][:, s]).astype(BF16NP)
        m["d0r"] = np.ascontiguousarray(desc0[b][:, s], f)
        m["d1r"] = np.ascontiguousarray(desc1[b][:, s], f)
        in_maps.append(m)
    return in_maps


_NC_CACHE = None


def kernel(desc0, desc1, Wq, bq, Wk, bk, Wv, bv, Wm, bm, W1, b1, ln_g, ln_b, W2, b2,
           trace=False):
    global _NC_CACHE
    desc0 = np.asarray(desc0, np.float32)
    desc1 = np.asarray(desc1, np.float32)
    weights = _prep_weights(
        np.asarray(Wq, np.float32), np.asarray(bq, np.float32),
        np.asarray(Wk, np.float32), np.asarray(bk, np.float32),
        np.asarray(Wv, np.float32), np.asarray(bv, np.float32),
        np.asarray(Wm, np.float32), np.asarray(bm, np.float32),
        np.asarray(W1, np.float32), np.asarray(b1, np.float32),
        np.asarray(ln_g, np.float32), np.asarray(ln_b, np.float32),
        np.asarray(W2, np.float32), np.asarray(b2, np.float32),
    )
    if _NC_CACHE is None:
        _NC_CACHE = build_program()
    nc = _NC_CACHE
    in_maps = make_in_maps(desc0, desc1, weights)
    res = run_bass_kernel_spmd(nc, in_maps, core_ids=list(range(N_CORES)), trace=trace)
    B = desc0.shape[0]
    out0 = np.empty((B, D, N), np.float32)
    out1 = np.empty((B, D, N), np.float32)
    for cid in range(N_CORES):
        b, j = cid // 4, cid % 4
        s = slice(j * NB, (j + 1) * NB)
        out0[b][:, s] = res.results[cid]["o0"]
        out1[b][:, s] = res.results[cid]["o1"]
    if trace:
        kernel.last_exec_time_ns = res.exec_time_ns
    return out0, out1
